# revision 1
# baseline (speedup 1.0000x reference)
"""MQA self-attention kernel for Trainium2, 8 NeuronCores.

Reference computation (fp32):
    q = x @ wq.T + bq        -> [B,S,1024] -> heads via (hidden num_heads) split
    k = x @ wk.T + bk        -> [B,S,64]  (single shared KV head)
    v = x @ wv.T + bv
    scores = q @ k.T / 8 ; attn = softmax(scores) ; h = attn @ v
    out = merge_heads(h) @ wo.T + bo

Sharding (8 cores, no collectives): core c handles batch b=c//4 and head
group g=c%4 (4 of the 16 q-heads).  The shared K/V head is replicated.
Each core returns the partial output h_g @ wo_g.T [S, D]; the host sums
the 4 head-group partials per batch and adds the bias terms.

Math notes:
 - bk provably cancels in softmax (adds a per-row constant to scores).
 - bv is folded into the output bias on host: softmax rows sum to 1, so
   attn @ (v + bv) = attn @ v + bv, contributing wo @ tile(bv, 16).
 - softmax is computed without max subtraction (scores ~ N(0,1); exp is
   safe in fp32) which lets exp(scores) @ [V|1] accumulate both the
   numerator and denominator in one PSUM pass.

Device layout (everything transposed so no on-device input transposes):
 - xT [1024, 2048] (d on partitions) is the rhs for all projections.
 - QT[q,s] and KT[d,s] computed directly in transposed layout; both are
   duplicated across SBUF partition halves so the scores matmul (K=64)
   runs as 64x128 row-tiled pairs over two key tiles at once.
 - scoresT[sk,sq] = KT.T @ QT per head; exp on ScalarE in [128,1024]
   blocks (amortizes the per-ACTIVATE overhead), PV matmul with V'=[V|1]
   (M=65) gives hT_un[d,sq] plus the softmax denominator in the same
   PSUM accumulation.
 - normalize via DVE reciprocal + broadcast-DMA + DVE multiply.
 - out partial = hT.T @ woT via PSUM accumulation over the 256 q dims.
All matmuls run in float32r (TF32-like, full PE rate at N=512).
"""

import numpy as np

NUM_HEADS = 16
Dh = 64
B, S, D = 2, 2048, 1024
G = 4            # head groups (cores per batch)
HG = 4           # heads per group
QD = HG * Dh     # 256 local q dims
NK = D // 128    # 8 contraction tiles for projections
NSK = S // 128   # 16 key tiles
W = 512          # matmul moving width
BLK = 1024       # sq block width for exp
NB = S // BLK    # 2 blocks
N_CORES = 8

_CACHE = {}


def _build_nc():
    from contextlib import ExitStack

    import concourse.bass as bass
    import concourse.mybir as mybir
    import concourse.tile as tile
    from concourse import bacc
    from concourse.masks import make_identity

    F32 = mybir.dt.float32
    F32R = mybir.dt.float32r
    EXP = mybir.ActivationFunctionType.Exp

    nc = bacc.Bacc("TRN2", target_bir_lowering=False, debug=False)

    xT = nc.declare_dram_parameter("xT", [D, S], F32R, isOutput=False)
    wqT = nc.declare_dram_parameter("wqT", [D, QD], F32R, isOutput=False)
    wvkT = nc.declare_dram_parameter("wvkT", [D, 128], F32R, isOutput=False)
    woT = nc.declare_dram_parameter("woT", [QD, D], F32R, isOutput=False)
    bqp = nc.declare_dram_parameter("bq", [QD, 1], F32, isOutput=False)
    part = nc.declare_dram_parameter("part", [S, D], F32, isOutput=True)

    with tile.TileContext(nc) as tc, ExitStack() as ctx:
        const = ctx.enter_context(tc.tile_pool(name="const", bufs=1))
        persist = ctx.enter_context(tc.tile_pool(name="persist", bufs=1))

        wq_sb = const.tile([128, NK * QD], F32R)    # ktile kt at cols [kt*QD:+QD]
        wvk_sb = const.tile([128, NK * 128], F32R)  # cols 0:64 of each ktile = wvT, 64:128 = wkT
        wo_sb = const.tile([128, 2 * D], F32R)      # q-ktile p at cols [p*D:+D]
        bq_sb = const.tile([128, 2], F32)
        ident = const.tile([128, 128], F32)
        ones_sb = const.tile([128, 1], F32)

        # qtd: per-head QT duplicated across both partition halves:
        # head h at cols [h*S:+S], rows 0:64 == rows 64:128 == QT_h [64, S]
        qtd_sb = persist.tile([128, HG * S], F32R)
        qod_sb = persist.tile([128, 2 * S], F32R)   # odd heads' QT at rows 0:64
        ktd_sb = persist.tile([128, S], F32R)       # KT staging (rows 64:128)
        kt0_sb = persist.tile([128, S], F32R)       # KT at rows 0:64
        v1_sb = persist.tile([128, NSK * 65], F32R)  # V' tile sk at cols [sk*65:+65]
        ht_sb = persist.tile([128, 2 * S], F32R)    # normalized hT, q-ktile p at cols [p*S:+S]

        make_identity(nc, ident[:])
        nc.vector.memset(ones_sb[:], 1.0)

        # ---- Phase 1: projections -------------------------------------
        with tc.tile_pool(name="xp", bufs=1) as xp:
            x_sb = xp.tile([128, NK * S], F32R)     # xT ktile kt at cols [kt*S:+S]
            vt_sb = xp.tile([128, S], F32)          # VT in rows 0:64
            for kt in range(NK):
                nc.sync.dma_start(wvk_sb[:, kt * 128:(kt + 1) * 128], wvkT[kt * 128:(kt + 1) * 128, :])
            for kt in range(NK):
                nc.sync.dma_start(wq_sb[:, kt * QD:(kt + 1) * QD], wqT[kt * 128:(kt + 1) * 128, :])
            for kt in range(NK):
                nc.sync.dma_start(x_sb[:, kt * S:(kt + 1) * S], xT[kt * 128:(kt + 1) * 128, :])
            for p in range(2):
                nc.sync.dma_start(bq_sb[:, p:p + 1], bqp[p * 128:(p + 1) * 128, :])
                nc.sync.dma_start(wo_sb[:, p * D:(p + 1) * D], woT[p * 128:(p + 1) * 128, :])

            # fused [V|K] projection interleaved with QT heads 0/1 so both
            # finish right as the last x tile lands.
            with (
                tc.tile_pool(name="vkps", bufs=1, space="PSUM") as vkps,
                tc.tile_pool(name="qps", bufs=1, space="PSUM") as qps,
            ):
                vk_ps = vkps.tile([128, S], F32)
                q_ps = qps.tile([128, S], F32)
                for kt in range(NK):
                    for n in range(S // W):
                        nc.tensor.matmul(
                            vk_ps[:, n * W:(n + 1) * W],
                            lhsT=wvk_sb[:, kt * 128:(kt + 1) * 128],
                            rhs=x_sb[:, kt * S + n * W: kt * S + (n + 1) * W],
                            start=(kt == 0), stop=(kt == NK - 1),
                        )
                    for n in range(S // W):
                        nc.tensor.matmul(
                            q_ps[:, n * W:(n + 1) * W],
                            lhsT=wq_sb[:, kt * QD: kt * QD + 128],
                            rhs=x_sb[:, kt * S + n * W: kt * S + (n + 1) * W],
                            start=(kt == 0), stop=(kt == NK - 1),
                        )
                # evacs split across DVE and the (still idle) ScalarE so the
                # path to the first scores matmul is short
                nc.vector.tensor_scalar_add(
                    qtd_sb[0:64, 0:S], q_ps[0:64, :], bq_sb[0:64, 0:1]
                )
                nc.scalar.copy(ktd_sb[64:128, :], vk_ps[64:128, :])
                nc.gpsimd.tensor_copy(kt0_sb[0:64, :], ktd_sb[64:128, :])
                nc.scalar.copy(vt_sb[0:64, :], vk_ps[0:64, :])
                nc.vector.tensor_scalar_add(
                    qtd_sb[64:128, S:2 * S], q_ps[64:128, :], bq_sb[64:128, 0:1]
                )
                nc.gpsimd.tensor_copy(qod_sb[0:64, 0:S], qtd_sb[64:128, S:2 * S])

            # QT heads 2/3 (reuses the freed PSUM banks)
            with tc.tile_pool(name="qps2", bufs=1, space="PSUM") as qps2:
                q_ps2 = qps2.tile([128, S], F32)
                for kt in range(NK):
                    for n in range(S // W):
                        nc.tensor.matmul(
                            q_ps2[:, n * W:(n + 1) * W],
                            lhsT=wq_sb[:, kt * QD + 128: kt * QD + 256],
                            rhs=x_sb[:, kt * S + n * W: kt * S + (n + 1) * W],
                            start=(kt == 0), stop=(kt == NK - 1),
                        )
                nc.vector.tensor_scalar_add(
                    qtd_sb[0:64, 2 * S:3 * S], q_ps2[0:64, :], bq_sb[0:64, 1:2]
                )
                nc.vector.tensor_scalar_add(
                    qtd_sb[64:128, 3 * S:4 * S], q_ps2[64:128, :], bq_sb[64:128, 1:2]
                )
                nc.gpsimd.tensor_copy(qod_sb[0:64, S:2 * S], qtd_sb[64:128, 3 * S:4 * S])

            # V' tiles: PE-transpose VT -> V[sk] = [128, 64], plus ones column
            with tc.tile_pool(name="trps", bufs=2, space="PSUM") as trps:
                for sk in range(NSK):
                    tr_ps = trps.tile([128, Dh], F32)
                    nc.tensor.transpose(
                        tr_ps[:], vt_sb[0:64, sk * 128:(sk + 1) * 128], ident[0:64, 0:64]
                    )
                    nc.vector.tensor_copy(v1_sb[:, sk * 65: sk * 65 + 64], tr_ps[:])
                    nc.vector.tensor_copy(v1_sb[:, sk * 65 + 64: sk * 65 + 65], ones_sb[:])

        # ---- Phase 2/3: attention + output projection ------------------
        # Software-pipelined at key-tile granularity: scores(sk) -> exp(sk)
        # -> PV(sk-2), so ScalarE (the exp bottleneck) never idles.
        with (
            tc.tile_pool(name="expp", bufs=6) as expp,
            tc.tile_pool(name="scps", bufs=2, space="PSUM") as scps,
            tc.tile_pool(name="pvps", bufs=1, space="PSUM") as pvps,
            tc.tile_pool(name="outps", bufs=2, space="PSUM") as outps,
            tc.tile_pool(name="smalls", bufs=4) as smalls,
            tc.tile_pool(name="bcp", bufs=4) as bcp,
            tc.tile_pool(name="osbp", bufs=3) as osbp,
        ):
            for b in range(NB):
                for h in range(HG):
                    qcol = (h if h % 2 == 0 else h // 2) * S + b * BLK
                    hv = [pvps.tile([128, W], F32, name=f"pv{half}")
                          for half in range(BLK // W)]
                    exp_tiles = [None] * NSK

                    def emit_pv(sk):
                        for half in range(BLK // W):
                            nc.tensor.matmul(
                                hv[half][0:65, :],
                                lhsT=v1_sb[:, sk * 65:(sk + 1) * 65],
                                rhs=exp_tiles[sk][:, half * W:(half + 1) * W],
                                start=(sk == 0), stop=(sk == NSK - 1),
                            )

                    for sk in range(NSK):
                        if sk >= 2:
                            emit_pv(sk - 2)
                        sc = scps.tile([128, BLK], F32, name="sc")
                        qsrc = qtd_sb if h % 2 == 0 else qod_sb
                        for n in range(BLK // W):
                            nc.tensor.matmul(
                                sc[:, n * W:(n + 1) * W],
                                lhsT=kt0_sb[0:64, sk * 128:(sk + 1) * 128],
                                rhs=qsrc[0:64, qcol + n * W: qcol + (n + 1) * W],
                                start=True, stop=True,
                            )
                        et = expp.tile([128, BLK], F32R, name="expt")
                        nc.scalar.activation(et[:], sc[:], EXP, scale=0.125)
                        exp_tiles[sk] = et
                    emit_pv(NSK - 2)
                    emit_pv(NSK - 1)

                    # normalize: hT[:, sq] /= sumexp[sq]
                    for half in range(BLK // W):
                        rec = smalls.tile([128, W], F32, name="rec")
                        nc.vector.reciprocal(rec[64:65, :], hv[half][64:65, :])
                        rec0 = smalls.tile([128, W], F32, name="rec0")
                        nc.gpsimd.tensor_copy(rec0[0:1, :], rec[64:65, :])
                        bc = bcp.tile([128, W], F32, name="bc")
                        nc.gpsimd.partition_broadcast(bc[0:64, :], rec0[0:1, :], channels=64)
                        hcol = (h // 2) * S + b * BLK + half * W
                        if h % 2 == 0:
                            nc.vector.tensor_mul(
                                ht_sb[0:64, hcol:hcol + W], hv[half][0:64, :], bc[0:64, :]
                            )
                        else:
                            tmp = bcp.tile([128, W], F32R, name="tmp")
                            nc.vector.tensor_mul(tmp[0:64, :], hv[half][0:64, :], bc[0:64, :])
                            # odd head lives in ht rows 64:128 (GpSimd partition shift)
                            nc.gpsimd.tensor_copy(ht_sb[64:128, hcol:hcol + W], tmp[0:64, :])

                # output projection for the 8 s-chunks of this block
                for sc_i in range(BLK // 128):
                    s = b * (BLK // 128) + sc_i
                    for n in range(2):
                        o_ps = outps.tile([128, W], F32, name="ops")
                        for p in range(2):
                            nc.tensor.matmul(
                                o_ps[:],
                                lhsT=ht_sb[:, p * S + s * 128: p * S + (s + 1) * 128],
                                rhs=wo_sb[:, p * D + n * W: p * D + (n + 1) * W],
                                start=(p == 0), stop=(p == 1),
                            )
                        o_sb = osbp.tile([128, W], F32, name="osb")
                        nc.vector.tensor_copy(o_sb[:], o_ps[:])
                        nc.sync.dma_start(
                            part[s * 128:(s + 1) * 128, n * W:(n + 1) * W], o_sb[:]
                        )

    nc.finalize()
    return nc


def _get_nc():
    if "nc" not in _CACHE:
        _CACHE["nc"] = _build_nc()
    return _CACHE["nc"]


def _prep_core_inputs(inputs, wq, bq, wk, wv, wo):
    """Host-side shard prep: per-core transposed/rearranged operands."""
    xT = [np.ascontiguousarray(np.asarray(inputs[b], np.float32).T) for b in range(B)]
    wq3 = np.asarray(wq, np.float32).reshape(Dh, NUM_HEADS, D)
    bq2 = np.asarray(bq, np.float32).reshape(Dh, NUM_HEADS)
    wvkT = np.ascontiguousarray(
        np.concatenate([np.asarray(wv, np.float32).T, np.asarray(wk, np.float32).T], axis=1)
    )  # [1024, 128]
    wo_ = np.asarray(wo, np.float32)

    in_maps = []
    for c in range(N_CORES):
        b, g = divmod(c, G)
        heads = [g * HG + hl for hl in range(HG)]
        # wqT_g [1024, 256]: column block hl = head (g*HG+hl), rows = d
        wqT_g = np.ascontiguousarray(
            np.concatenate([wq3[:, h, :].T for h in heads], axis=1)
        )
        bq_g = np.ascontiguousarray(
            np.concatenate([bq2[:, h] for h in heads]).reshape(QD, 1)
        )
        woT_g = np.ascontiguousarray(wo_[:, g * QD:(g + 1) * QD].T)  # [256, 1024]
        in_maps.append({
            "xT": xT[b],
            "wqT": wqT_g,
            "wvkT": wvkT,
            "woT": woT_g,
            "bq": bq_g,
        })
    return in_maps


def kernel(inputs, wq, bq, wk, bk, wv, bv, wo, bo):
    from concourse.bass_utils import run_bass_kernel_spmd

    nc = _get_nc()
    in_maps = _prep_core_inputs(inputs, wq, bq, wk, wv, wo)
    res = run_bass_kernel_spmd(nc, in_maps, list(range(N_CORES))).results

    wo_ = np.asarray(wo, np.float32)
    bias = (
        np.asarray(bo, np.float32)
        + wo_ @ np.tile(np.asarray(bv, np.float32), NUM_HEADS)
    )
    out = np.empty((B, S, D), np.float32)
    for b in range(B):
        acc = res[b * G]["part"].astype(np.float32).copy()
        for g in range(1, G):
            acc += res[b * G + g]["part"]
        out[b] = acc + bias
    return out



# revision 3
# speedup vs baseline: 1.2281x; 1.2281x over previous
"""MQA self-attention kernel for Trainium2, 8 NeuronCores.

Reference computation (fp32):
    q = x @ wq.T + bq        -> [B,S,1024] -> heads via (hidden num_heads) split
    k = x @ wk.T + bk        -> [B,S,64]  (single shared KV head)
    v = x @ wv.T + bv
    scores = q @ k.T / 8 ; attn = softmax(scores) ; h = attn @ v
    out = merge_heads(h) @ wo.T + bo

Sharding (8 cores, no collectives): core c handles batch b=c//4 and head
group g=c%4 (4 of the 16 q-heads).  The shared K/V head is replicated.
Each core returns the partial output h_g @ wo_g.T [S, D]; the host sums
the 4 head-group partials per batch and adds the bias terms.

Per-core schedule (PE-bound at ~280k cycles):
 - Projections in fp32r (full PE rate at N=512).  K lands in psum rows
   0:64 (wvkT = [wk.T | wv.T]) so its evac needs no partition shift; the
   KT row block is duplicated to partitions 64:128 so odd heads' scores
   matmuls can run against QT stored in the other partition half.
 - scoresT[sk,sq] = KT.T @ QT per head in [128,512] psum tiles.
 - exp is split across two engines: ACT runs the real Exp activation
   (bf16 out); DVE computes a Schraudolph-style exp2 via one
   tensor_scalar (x*A+B -> int16, bitcast bf16).  Softmax renormalizes,
   so the ~3% sawtooth error mostly cancels; measured end-to-end rel
   err ~1e-2 vs the 2e-2 gate.
 - PV is flipped: the exp tile [sk 128, sq 128] is the stationary
   operand and V' = [V | 1] [128, 65] moves, so each matmul costs 65
   moving rows instead of 512 (PE charges by moving dim only).  The
   ones column accumulates the softmax denominator in the same psum
   tile ([sq, 65], col 64 = sum).
 - normalize on evac: DVE reciprocal of the strided denominators, then
   ACT Copy-with-per-partition-scale (and DVE tensor_scalar) write
   normalized bf16 h for a head pair packed [sq, h_even|h_odd].
 - DMA-transpose (free XBAR engine) flips each [sq 128, 128] pair block
   into ht [128 qd, sq] for the out projection (bf16, fp32 accum).
"""

import numpy as np

NUM_HEADS = 16
Dh = 64
B, S, D = 2, 2048, 1024
G = 4            # head groups (cores per batch)
HG = 4           # heads per group
QD = HG * Dh     # 256 local q dims
NK = D // 128    # 8 contraction tiles for projections
NSK = S // 128   # 16 key tiles
W = 512          # matmul moving width
NB = 2           # sq blocks of 1024
BLK = 1024
N_CORES = 8

# Schraudolph exp2 constants (bf16 bit domain), score scale 1/8 folded in.
EXP_A = float(128.0 / np.log(2.0) * 0.125)
EXP_B = float(127.0 * 128.0 - 128.0 * np.log2(1.03279))

# exp engine split per (sk, half): True -> DVE schraudolph, else ACT Exp.
DVE_SK0 = frozenset({2, 5, 8, 11, 14})
DVE_SK1 = frozenset({0, 2, 4, 6, 8, 10, 12, 14})
# normalize-mul engine split per sq-tile: True -> DVE
DVE_MUL = frozenset({3, 7})

_CACHE = {}


def _build_nc():
    from contextlib import ExitStack

    import concourse.bass as bass
    import concourse.mybir as mybir
    import concourse.tile as tile
    from concourse import bacc
    from concourse.masks import make_identity

    F32 = mybir.dt.float32
    F32R = mybir.dt.float32r
    BF16 = mybir.dt.bfloat16
    I16 = mybir.dt.int16
    EXP = mybir.ActivationFunctionType.Exp
    COPY = mybir.ActivationFunctionType.Copy
    MUL = mybir.AluOpType.mult
    ADD = mybir.AluOpType.add

    nc = bacc.Bacc("TRN2", target_bir_lowering=False, debug=False)

    xT = nc.declare_dram_parameter("xT", [D, S], F32R, isOutput=False)
    wqT = nc.declare_dram_parameter("wqT", [D, QD], F32R, isOutput=False)
    wvkT = nc.declare_dram_parameter("wvkT", [D, 128], F32R, isOutput=False)
    woT = nc.declare_dram_parameter("woT", [QD, D], BF16, isOutput=False)
    bqp = nc.declare_dram_parameter("bq", [128, 2], F32, isOutput=False)
    part = nc.declare_dram_parameter("part", [S, D], F32, isOutput=True)

    with tile.TileContext(nc) as tc, ExitStack() as ctx:
        const = ctx.enter_context(tc.tile_pool(name="const", bufs=1))
        persist = ctx.enter_context(tc.tile_pool(name="persist", bufs=1))

        wq_sb = const.tile([128, NK * QD], F32R)    # ktile kt at cols [kt*QD:+QD]
        wvk_sb = const.tile([128, NK * 128], F32R)  # cols 0:64 = wkT, 64:128 = wvT
        wo_sb = const.tile([128, 2 * D], BF16)      # qd-ktile p at cols [p*D:+D]
        bq_sb = const.tile([128, 2], F32)
        ident = const.tile([128, 128], F32)

        kt_sb = persist.tile([128, S], F32R)        # KT dup in both partition halves
        qt_sb = persist.tile([128, 2 * S], F32R)    # pass p: heads (2p,2p+1) stacked
        v1_sb = persist.tile([128, NSK * 65], BF16)  # V' tile sk at cols [sk*65:+65]
        ht_sb = persist.tile([128, 2 * BLK], BF16)  # block-local hT, pair p at [p*BLK:+BLK]

        make_identity(nc, ident[:])
        nc.vector.memset(v1_sb[:, 64:NSK * 65:65], 1.0)

        # ---- Phase 0/1: DMAs + projections ----------------------------
        with tc.tile_pool(name="xp", bufs=1) as xp:
            x_sb = xp.tile([128, NK * S], F32R)     # xT ktile kt at cols [kt*S:+S]
            vt_sb = xp.tile([128, S], F32)          # VT in rows 64:128

            # weights first (small), then x tiles split across SP/Pool queues
            for kt in range(NK):
                nc.sync.dma_start(wvk_sb[:, kt * 128:(kt + 1) * 128], wvkT[kt * 128:(kt + 1) * 128, :])
            nc.sync.dma_start(bq_sb[:], bqp[:, :])
            for kt in range(NK):
                nc.gpsimd.dma_start(wq_sb[:, kt * QD:(kt + 1) * QD], wqT[kt * 128:(kt + 1) * 128, :])
            for kt in range(NK):
                eng = nc.sync if kt % 2 == 0 else nc.gpsimd
                eng.dma_start(x_sb[:, kt * S:(kt + 1) * S], xT[kt * 128:(kt + 1) * 128, :])
            for p in range(2):
                nc.sync.dma_start(wo_sb[:, p * D:(p + 1) * D], woT[p * 128:(p + 1) * 128, :])

            with (
                tc.tile_pool(name="vkps", bufs=1, space="PSUM") as vkps,
                tc.tile_pool(name="qps", bufs=1, space="PSUM") as qps,
            ):
                vk_ps = [vkps.tile([128, BLK], F32, name=f"vk{h}") for h in range(2)]
                q_ps = [qps.tile([128, BLK], F32, name=f"q{h}") for h in range(2)]
                for kt in range(NK):
                    for h in range(2):
                        for n in range(2):
                            nc.tensor.matmul(
                                vk_ps[h][:, n * W:(n + 1) * W],
                                lhsT=wvk_sb[:, kt * 128:(kt + 1) * 128],
                                rhs=x_sb[:, kt * S + h * BLK + n * W: kt * S + h * BLK + (n + 1) * W],
                                start=(kt == 0), stop=(kt == NK - 1),
                            )
                        for n in range(2):
                            nc.tensor.matmul(
                                q_ps[h][:, n * W:(n + 1) * W],
                                lhsT=wq_sb[:, kt * QD: kt * QD + 128],
                                rhs=x_sb[:, kt * S + h * BLK + n * W: kt * S + h * BLK + (n + 1) * W],
                                start=(kt == 0), stop=(kt == NK - 1),
                            )
                # evacs: KT + VT on ACT, Q pass0 (+bias) on DVE
                for h in range(2):
                    nc.scalar.copy(kt_sb[0:64, h * BLK:(h + 1) * BLK], vk_ps[h][0:64, :])
                    nc.scalar.copy(vt_sb[64:128, h * BLK:(h + 1) * BLK], vk_ps[h][64:128, :])
                    nc.vector.tensor_scalar_add(
                        qt_sb[:, h * BLK:(h + 1) * BLK], q_ps[h][:, :], bq_sb[:, 0:1]
                    )
                nc.gpsimd.tensor_copy(kt_sb[64:128, :], kt_sb[0:64, :])

            # Q pass 1 (heads 2,3) reuses freed banks; V' transposes after
            with (
                tc.tile_pool(name="qps2", bufs=1, space="PSUM") as qps2,
                tc.tile_pool(name="trps", bufs=2, space="PSUM") as trps,
            ):
                q_ps2 = [qps2.tile([128, BLK], F32, name=f"q2{h}") for h in range(2)]
                for kt in range(NK):
                    for h in range(2):
                        for n in range(2):
                            nc.tensor.matmul(
                                q_ps2[h][:, n * W:(n + 1) * W],
                                lhsT=wq_sb[:, kt * QD + 128: kt * QD + 256],
                                rhs=x_sb[:, kt * S + h * BLK + n * W: kt * S + h * BLK + (n + 1) * W],
                                start=(kt == 0), stop=(kt == NK - 1),
                            )
                for sk in range(NSK):
                    tr_ps = trps.tile([128, Dh], F32, name="trp")
                    nc.tensor.transpose(
                        tr_ps[:], vt_sb[64:128, sk * 128:(sk + 1) * 128],
                        ident[64:128, 64:128],
                    )
                    nc.scalar.copy(v1_sb[:, sk * 65: sk * 65 + 64], tr_ps[:])
                for h in range(2):
                    nc.vector.tensor_scalar_add(
                        qt_sb[:, S + h * BLK: S + (h + 1) * BLK], q_ps2[h][:, :], bq_sb[:, 1:2]
                    )

        # ---- Phase 2: attention + output projection --------------------
        with (
            tc.tile_pool(name="expp", bufs=8) as expp,
            tc.tile_pool(name="scps", bufs=4, space="PSUM") as scps,
            tc.tile_pool(name="hps", bufs=1, space="PSUM") as hps,
            tc.tile_pool(name="outps", bufs=2, space="PSUM") as outps,
            tc.tile_pool(name="smalls", bufs=2) as smalls,
            tc.tile_pool(name="hnp", bufs=2) as hnp,
            tc.tile_pool(name="osbp", bufs=3) as osbp,
        ):
            h_ps = [hps.tile([128, W], F32, name=f"hb{i}") for i in range(2)]
            for blk in range(NB):
                hn_t = None
                for hl in range(HG):
                    half, pair = hl % 2, hl // 2
                    rows = slice(0, 64) if half == 0 else slice(64, 128)
                    qcol = pair * S + blk * BLK
                    exp_tiles = [None] * NSK

                    def emit_pv(sk):
                        # start/stop bracket the whole bank's accumulation
                        # group (4 interleaved [*,65] regions share one 2KB
                        # zero region; lazy zeroing covers regions 1-3).
                        et0, et1 = exp_tiles[sk]
                        for t in range(8):
                            et = et0 if t < 4 else et1
                            c = t % 4
                            nc.tensor.matmul(
                                h_ps[t // 4][:, c * 65: c * 65 + 65],
                                lhsT=et[:, c * 128:(c + 1) * 128],
                                rhs=v1_sb[:, sk * 65:(sk + 1) * 65],
                                start=(sk == 0 and c == 0),
                                stop=(sk == NSK - 1 and c == 3),
                            )

                    for sk in range(NSK):
                        ets = []
                        for n in range(2):
                            sc = scps.tile([128, W], F32, name="sc")
                            nc.tensor.matmul(
                                sc[:],
                                lhsT=kt_sb[rows, sk * 128:(sk + 1) * 128],
                                rhs=qt_sb[rows, qcol + n * W: qcol + (n + 1) * W],
                                start=True, stop=True,
                            )
                            et = expp.tile([128, W], BF16, name="et")
                            if sk in (DVE_SK0 if n == 0 else DVE_SK1):
                                nc.vector.tensor_scalar(
                                    et[:].bitcast(I16), sc[:], EXP_A, EXP_B, MUL, ADD
                                )
                            else:
                                nc.scalar.activation(et[:], sc[:], EXP, scale=0.125)
                            ets.append(et)
                        exp_tiles[sk] = ets
                        if sk >= 2:
                            emit_pv(sk - 2)
                    emit_pv(NSK - 2)
                    emit_pv(NSK - 1)

                    # normalize + pack head pair [sq, h_even|h_odd] in bf16
                    if half == 0:
                        hn_t = hnp.tile([128, 8 * 128], BF16, name="hn")
                    rec = smalls.tile([128, 8], F32, name="rec")
                    for i in range(2):
                        nc.vector.reciprocal(
                            rec[:, i * 4:(i + 1) * 4], h_ps[i][:, 64:260:65]
                        )
                    for t in range(8):
                        i, c = t // 4, t % 4
                        dst = hn_t[:, t * 128 + half * 64: t * 128 + half * 64 + 64]
                        if t in DVE_MUL:
                            nc.vector.tensor_scalar(
                                dst, h_ps[i][:, c * 65: c * 65 + 64],
                                rec[:, t:t + 1], None, MUL,
                            )
                        else:
                            nc.scalar.activation(
                                dst, h_ps[i][:, c * 65: c * 65 + 64],
                                COPY, scale=rec[:, t:t + 1],
                            )
                    if half == 1:
                        for t in range(8):
                            nc.sync.dma_start_transpose(
                                ht_sb[:, pair * BLK + t * 128: pair * BLK + (t + 1) * 128],
                                hn_t[:, t * 128:(t + 1) * 128],
                            )

                # output projection for this block's 8 s-chunks
                for s in range(8):
                    for n in range(2):
                        o_ps = outps.tile([128, W], F32, name="ops")
                        for p in range(2):
                            nc.tensor.matmul(
                                o_ps[:],
                                lhsT=ht_sb[:, p * BLK + s * 128: p * BLK + (s + 1) * 128],
                                rhs=wo_sb[:, p * D + n * W: p * D + (n + 1) * W],
                                start=(p == 0), stop=(p == 1),
                            )
                        o_sb = osbp.tile([128, W], F32, name="osb")
                        if (s + n) % 2 == 0:
                            nc.scalar.copy(o_sb[:], o_ps[:])
                        else:
                            nc.vector.tensor_copy(o_sb[:], o_ps[:])
                        nc.gpsimd.dma_start(
                            part[blk * BLK + s * 128: blk * BLK + (s + 1) * 128,
                                 n * W:(n + 1) * W],
                            o_sb[:],
                        )

    nc.finalize()
    return nc


def _get_nc():
    if "nc" not in _CACHE:
        _CACHE["nc"] = _build_nc()
    return _CACHE["nc"]


def _to_bf16_bits(a):
    """fp32 -> bf16 bits (round-to-nearest-even), as uint16."""
    u = np.asarray(a, np.float32).view(np.uint32)
    return ((u + 0x7FFF + ((u >> 16) & 1)) >> 16).astype(np.uint16)


def _prep_core_inputs(inputs, wq, bq, wk, wv, wo):
    """Host-side shard prep: per-core transposed/rearranged operands."""
    xT = [np.ascontiguousarray(np.asarray(inputs[b], np.float32).T) for b in range(B)]
    wq3 = np.asarray(wq, np.float32).reshape(Dh, NUM_HEADS, D)
    bq2 = np.asarray(bq, np.float32).reshape(Dh, NUM_HEADS)
    wvkT = np.ascontiguousarray(
        np.concatenate([np.asarray(wk, np.float32).T, np.asarray(wv, np.float32).T], axis=1)
    )  # [1024, 128], K first
    wo_ = np.asarray(wo, np.float32)

    in_maps = []
    for c in range(N_CORES):
        b, g = divmod(c, G)
        heads = [g * HG + hl for hl in range(HG)]
        wqT_g = np.ascontiguousarray(
            np.concatenate([wq3[:, h, :].T for h in heads], axis=1)
        )  # [1024, 256]
        bq_g = np.ascontiguousarray(
            np.concatenate([bq2[:, h] for h in heads]).reshape(2, 128).T
        )  # [128, 2]: col p = heads (2p, 2p+1) stacked
        woT_g = _to_bf16_bits(wo_[:, g * QD:(g + 1) * QD].T)  # [256, 1024] bf16
        in_maps.append({
            "xT": xT[b],
            "wqT": wqT_g,
            "wvkT": wvkT,
            "woT": woT_g,
            "bq": bq_g,
        })
    return in_maps


def kernel(inputs, wq, bq, wk, bk, wv, bv, wo, bo):
    from concourse.bass_utils import run_bass_kernel_spmd

    nc = _get_nc()
    in_maps = _prep_core_inputs(inputs, wq, bq, wk, wv, wo)
    res = run_bass_kernel_spmd(nc, in_maps, list(range(N_CORES))).results

    wo_ = np.asarray(wo, np.float32)
    bias = (
        np.asarray(bo, np.float32)
        + wo_ @ np.tile(np.asarray(bv, np.float32), NUM_HEADS)
    )
    out = np.empty((B, S, D), np.float32)
    for b in range(B):
        acc = res[b * G]["part"].astype(np.float32).copy()
        for g in range(1, G):
            acc += res[b * G + g]["part"]
        out[b] = acc + bias
    return out
